# revision 4
# baseline (speedup 1.0000x reference)
"""DCRNN decoder kernel for Trainium2, 8 NeuronCores, batch-data-parallel.

Strategy:
  - Shard batch 64 -> 8 cores x 8.  Supports/weights replicated. No collectives.
  - Feature-major layout on device: X[c, b*326 + n] (node dim padded 325->326
    to satisfy the fp32r even-free-dim ISA restriction).
  - Chebyshev recursion folded into 4 host-precomputed node operators
    T1=S1, T2=2*S1@S1-I, T3=S2, T4=2*S2@S2-I, so each gconv is:
      per-b PE transpose of x0 (feature-major -> node-major stationary),
      4 accumulating T-matmuls (mats), then a 5-term weight contraction
      plus a K=6 matmul for the layer-0 raw-input channel (precomputed
      on host as xm5[t] = [x_t, T1 x_t, ..., T4 x_t]).
  - All matmuls in float32r (TF32) with f32 PSUM accumulation.
"""
import sys

sys.path.insert(0, "/opt/trn_rl_repo")

import numpy as np

N, NP, HID = 325, 326, 64
B_L = 8          # batch per core
NCORES = 8
T_STEPS = 12
BNP = B_L * NP   # 2608
KS = [128, 128, 70]    # node chunks (last includes the zero pad row)
K0S = [0, 128, 256]

_CACHE = {}


def _build():
    if "nc" in _CACHE:
        return _CACHE["nc"]

    import concourse.bass as bass  # noqa: F401
    import concourse.mybir as mybir
    import concourse.tile as tile
    import concourse.bacc as bacc

    f32 = mybir.dt.float32
    f32r = mybir.dt.float32r
    AF = mybir.ActivationFunctionType

    nc = bacc.Bacc("TRN2", target_bir_lowering=False, debug=False,
                   num_devices=NCORES)

    def din(name, shape, dt=f32r):
        return nc.dram_tensor(name, shape, dt, kind="ExternalInput").ap()

    tT_d = din("tT", [128, 12 * NP])
    eye_d = din("eye", [128, 128])
    wg0h_d = din("wg0h", [64, 640])
    wg0x_d = din("wg0x", [6, 128])
    wc0h_d = din("wc0h", [64, 320])
    wc0x_d = din("wc0x", [6, 64])
    wg1_d = din("wg1", [128, 640])
    wc1_d = din("wc1", [128, 320])
    wfc_d = din("wfc", [64, 2])
    bg0_d = din("bg0", [128, 1], f32)
    bc0_d = din("bc0", [64, 1], f32)
    bg1_d = din("bg1", [128, 1], f32)
    bc1_d = din("bc1", [64, 1], f32)
    xm5_d = din("xm5", [T_STEPS, 6, BNP])
    h0i_d = din("h0i", [64, BNP])
    h1i_d = din("h1i", [64, BNP])
    out_d = nc.dram_tensor("out", [T_STEPS, BNP], f32, kind="ExternalOutput").ap()

    with tile.TileContext(nc) as tc:
        import contextlib
        with contextlib.ExitStack() as ctx:
            const = ctx.enter_context(tc.tile_pool(name="const", bufs=1))
            xm5p = ctx.enter_context(tc.tile_pool(name="xm5p", bufs=1))
            h0p = ctx.enter_context(tc.tile_pool(name="h0p", bufs=2))
            h1p = ctx.enter_context(tc.tile_pool(name="h1p", bufs=2))
            x0g1p = ctx.enter_context(tc.tile_pool(name="x0g1p", bufs=2))
            x0c0p = ctx.enter_context(tc.tile_pool(name="x0c0p", bufs=2))
            x0c1p = ctx.enter_context(tc.tile_pool(name="x0c1p", bufs=2))
            x0Tp = ctx.enter_context(tc.tile_pool(name="x0Tp", bufs=6))
            matsp = ctx.enter_context(tc.tile_pool(name="matsp", bufs=4))
            rufp = ctx.enter_context(tc.tile_pool(name="rufp", bufs=1))
            psT = ctx.enter_context(tc.tile_pool(name="psT", bufs=2, space="PSUM"))
            psS = ctx.enter_context(tc.tile_pool(name="psS", bufs=3, space="PSUM"))
            psW = ctx.enter_context(tc.tile_pool(name="psW", bufs=2, space="PSUM"))
            psF = ctx.enter_context(tc.tile_pool(name="psF", bufs=1, space="PSUM"))

            def load(name, src, shape, dt=f32r):
                t = const.tile(shape, dt, tag=name)
                nc.sync.dma_start(t[:], src[:])
                return t

            tT = load("tT", tT_d, [128, 12 * NP])
            eye = load("eye", eye_d, [128, 128])
            wg0h = load("wg0h", wg0h_d, [64, 640])
            wg0x = load("wg0x", wg0x_d, [6, 128])
            wc0h = load("wc0h", wc0h_d, [64, 320])
            wc0x = load("wc0x", wc0x_d, [6, 64])
            wg1 = load("wg1", wg1_d, [128, 640])
            wc1 = load("wc1", wc1_d, [128, 320])
            wfc = load("wfc", wfc_d, [64, 2])
            bg0 = load("bg0", bg0_d, [128, 1], f32)
            bc0 = load("bc0", bc0_d, [64, 1], f32)
            bg1 = load("bg1", bg1_d, [128, 1], f32)
            bc1 = load("bc1", bc1_d, [64, 1], f32)

            h0_t = h0p.tile([64, BNP], f32r, tag="h0")
            nc.sync.dma_start(h0_t[:], h0i_d[:])
            h1_t = h1p.tile([64, BNP], f32r, tag="h1")
            nc.sync.dma_start(h1_t[:], h1i_d[:])

            evac_ctr = [0]

            def evac(dst, src):
                """PSUM -> SBUF copy alternating between ACT and DVE."""
                evac_ctr[0] += 1
                if evac_ctr[0] % 2 == 0:
                    nc.scalar.activation(dst, src, AF.Copy)
                else:
                    nc.vector.tensor_copy(dst, src)

            def gconv(src, C, wh, O, wx, xm5_t, bias, act, out_r, out_u):
                """One graph convolution.
                src:  [C, BNP] f32r feature-major input tile
                wh:   [C, 5*O] packed h-part weights
                wx:   [6, O] x-part weights or None; xm5_t the matching rhs
                act:  Sigmoid (gates, writes out_r/out_u [64,*] halves) or
                      Tanh (candidate, writes out_r only, O=64)
                """
                x0Ts = []
                for b in range(B_L):
                    pt = psT.tile([128, 3 * C], f32r, tag="psT")
                    for k in range(3):
                        nc.tensor.transpose(
                            pt[0:KS[k], k * C:(k + 1) * C],
                            src[:, b * NP + K0S[k]: b * NP + K0S[k] + KS[k]],
                            eye[0:C, 0:C])
                    x0T = x0Tp.tile([128, 3 * C], f32r, tag="x0T")
                    evac(x0T[:, :], pt[:, :])
                    x0Ts.append(x0T)
                mats = []
                for b in range(B_L):
                    mat = matsp.tile([C, 4 * NP], f32r, tag="mats")
                    for m in range(4):
                        pS = psS.tile([C, NP], f32, tag="psS")
                        for k in range(3):
                            nc.tensor.matmul(
                                pS[:, :],
                                x0Ts[b][0:KS[k], k * C:(k + 1) * C],
                                tT[0:KS[k], (m * 3 + k) * NP:(m * 3 + k + 1) * NP],
                                start=(k == 0), stop=(k == 2))
                        evac(mat[:, m * NP:(m + 1) * NP], pS[:, :])
                    mats.append(mat)
                for b in range(B_L):
                    bs = slice(b * NP, (b + 1) * NP)
                    pW = psW.tile([O, NP], f32, tag="psW")
                    for m in range(4):
                        nc.tensor.matmul(
                            pW[:, :], wh[:, (m + 1) * O:(m + 2) * O],
                            mats[b][:, m * NP:(m + 1) * NP],
                            start=(m == 0), stop=False)
                    last = wx is None
                    nc.tensor.matmul(pW[:, :], wh[:, 0:O], src[:, bs],
                                     start=False, stop=last)
                    if wx is not None:
                        nc.tensor.matmul(pW[:, :], wx[:, :],
                                         xm5_t[:, bs], start=False, stop=True)
                    if act == AF.Sigmoid:
                        nc.scalar.activation(out_r[0:64, bs], pW[0:64, :],
                                             AF.Sigmoid, bias=bias[0:64, 0:1])
                        nc.scalar.activation(out_u[0:64, bs], pW[64:128, :],
                                             AF.Sigmoid, bias=bias[64:128, 0:1])
                    else:
                        nc.scalar.activation(out_r[0:64, bs], pW[0:64, :],
                                             AF.Tanh, bias=bias[0:64, 0:1])

            for t in range(T_STEPS):
                xm5_t = xm5p.tile([6, BNP], f32r, tag="xm5")
                nc.sync.dma_start(xm5_t[:], xm5_d[t, :, :])

                # ---- layer 0 gates: gconv(x_t, h0)
                r_t = rufp.tile([64, BNP], f32, tag="rc")
                u_t = rufp.tile([64, BNP], f32, tag="u")
                gconv(h0_t, 64, wg0h, 128, wg0x, xm5_t, bg0, AF.Sigmoid, r_t, u_t)

                # r*h0 -> candidate input tile
                x0c0 = x0c0p.tile([64, BNP], f32r, tag="x0c0")
                for b in range(B_L):
                    bs = slice(b * NP, (b + 1) * NP)
                    nc.vector.tensor_mul(x0c0[:, bs], r_t[:, bs],
                                         h0_t[:, bs].bitcast(f32))

                # ---- layer 0 candidate: gconv(x_t, r*h0) -> tanh
                c_t = rufp.tile([64, BNP], f32, tag="rc")
                gconv(x0c0, 64, wc0h, 64, wc0x, xm5_t, bc0, AF.Tanh, c_t, None)

                # ---- cell update L0: h0n = u*h0 + (1-u)*c = c + u*(h0-c)
                h0_n = h0p.tile([64, BNP], f32r, tag="h0")
                x0g1 = x0g1p.tile([128, BNP], f32r, tag="x0g1")
                x0c1 = x0c1p.tile([128, BNP], f32r, tag="x0c1")
                for b in range(B_L):
                    bs = slice(b * NP, (b + 1) * NP)
                    nc.vector.tensor_sub(h0_n[:, bs], h0_t[:, bs].bitcast(f32),
                                         c_t[:, bs])
                    nc.vector.tensor_mul(x0c0[:, bs], u_t[:, bs],
                                         h0_n[:, bs].bitcast(f32))
                    nc.vector.tensor_add(h0_n[:, bs], x0c0[:, bs].bitcast(f32),
                                         c_t[:, bs])
                    nc.scalar.activation(x0g1[0:64, bs],
                                         h0_n[:, bs].bitcast(f32), AF.Copy)
                    nc.vector.tensor_copy(x0c1[0:64, bs], h0_n[:, bs])
                    nc.scalar.activation(x0g1[64:128, bs],
                                         h1_t[:, bs].bitcast(f32), AF.Copy)

                # ---- layer 1 gates: gconv(h0n, h1)
                r1_t = rufp.tile([64, BNP], f32, tag="rc")
                u1_t = rufp.tile([64, BNP], f32, tag="u")
                gconv(x0g1, 128, wg1, 128, None, None, bg1, AF.Sigmoid, r1_t, u1_t)

                for b in range(B_L):
                    bs = slice(b * NP, (b + 1) * NP)
                    nc.vector.tensor_mul(x0c1[64:128, bs], r1_t[:, bs],
                                         h1_t[:, bs].bitcast(f32))

                # ---- layer 1 candidate
                c1_t = rufp.tile([64, BNP], f32, tag="rc")
                gconv(x0c1, 128, wc1, 64, None, None, bc1, AF.Tanh, c1_t, None)

                # ---- cell update L1
                h1_n = h1p.tile([64, BNP], f32r, tag="h1")
                for b in range(B_L):
                    bs = slice(b * NP, (b + 1) * NP)
                    nc.vector.tensor_sub(h1_n[:, bs], h1_t[:, bs].bitcast(f32),
                                         c1_t[:, bs])
                    nc.vector.tensor_mul(x0c1[0:64, bs], u1_t[:, bs],
                                         h1_n[:, bs].bitcast(f32))
                    nc.vector.tensor_add(h1_n[:, bs], x0c1[0:64, bs].bitcast(f32),
                                         c1_t[:, bs])

                # ---- FC output: out[t] = W_fc.T @ h1n (bias added on host)
                # stage through the (now dead) u1 tile's partition row 0
                for b in range(B_L):
                    bs = slice(b * NP, (b + 1) * NP)
                    pF = psF.tile([1, NP], f32, tag="psF")
                    nc.tensor.matmul(pF[0:1, :], wfc[:, 0:1], h1_n[:, bs],
                                     start=True, stop=True)
                    nc.scalar.activation(u1_t[0:1, bs], pF[0:1, :], AF.Copy)
                nc.sync.dma_start(out_d[t:t + 1, :], u1_t[0:1, :])

                h0_t = h0_n
                h1_t = h1_n

    nc.compile()
    _CACHE["nc"] = nc
    return nc


def _host_prep(inputs):
    adj = np.asarray(inputs["adj_mx"], np.float64)
    I = np.eye(N)
    S1 = (adj / adj.sum(1, keepdims=True)).T
    adjT = adj.T
    S2 = (adjT / adjT.sum(1, keepdims=True)).T
    Ts = [I, S1, 2.0 * S1 @ S1 - I, S2, 2.0 * S2 @ S2 - I]
    Ts32 = [T.astype(np.float32) for T in Ts]

    tT = np.zeros((128, 12 * NP), np.float32)
    for m in range(4):
        Tm = Ts32[m + 1]
        for k in range(3):
            k0 = K0S[k]
            kk = min(KS[k], N - k0)
            tT[0:kk, (m * 3 + k) * NP:(m * 3 + k) * NP + N] = Tm[:, k0:k0 + kk].T

    def packw(W, C, O):
        # W [(1+C or C)*5, O] row index = c*5+m; returns h-part [C,5*O], x-part [6,O]|None
        Wr = np.asarray(W, np.float32)
        tot = Wr.shape[0] // 5
        Wr = Wr.reshape(tot, 5, O)
        if tot == C + 1:
            wh = Wr[1:].reshape(C, 5 * O)
            wx = np.concatenate([Wr[0], np.zeros((1, O), np.float32)], 0)
        else:
            wh = Wr.reshape(C, 5 * O)
            wx = None
        return wh, wx

    wg0h, wg0x = packw(inputs["Wg0"], 64, 128)
    wc0h, wc0x = packw(inputs["Wc0"], 64, 64)
    wg1, _ = packw(inputs["Wg1"], 128, 128)
    wc1, _ = packw(inputs["Wc1"], 128, 64)

    wfc = np.zeros((64, 2), np.float32)
    wfc[:, 0] = np.asarray(inputs["W_fc"], np.float32)[:, 0]

    common = {
        "tT": tT,
        "eye": np.eye(128, dtype=np.float32),
        "wg0h": wg0h, "wg0x": wg0x, "wc0h": wc0h, "wc0x": wc0x,
        "wg1": wg1, "wc1": wc1, "wfc": wfc,
        "bg0": np.asarray(inputs["bg0"], np.float32).reshape(128, 1),
        "bc0": np.asarray(inputs["bc0"], np.float32).reshape(64, 1),
        "bg1": np.asarray(inputs["bg1"], np.float32).reshape(128, 1),
        "bc1": np.asarray(inputs["bc1"], np.float32).reshape(64, 1),
    }

    x_all = np.asarray(inputs["inputs"], np.float32)[0:T_STEPS, :, :, 0]  # [12,64,325]
    init = np.asarray(inputs["init_state"], np.float32)  # [2, 64, 20800]

    in_maps = []
    for ci in range(NCORES):
        b0 = ci * B_L
        x = x_all[:, b0:b0 + B_L, :]                       # [12, 8, 325]
        xm5 = np.zeros((T_STEPS, 6, B_L, NP), np.float32)
        flat = x.reshape(T_STEPS * B_L, N)
        for m in range(5):
            xm = flat @ Ts32[m].T                          # [(t,b), n']
            xm5[:, m, :, 0:N] = xm.reshape(T_STEPS, B_L, N)
        xm5 = xm5.reshape(T_STEPS, 6, BNP)

        def fm(h):   # [8, 20800] -> [64, BNP]
            a = h.reshape(B_L, N, HID).transpose(2, 0, 1)  # [64, 8, 325]
            out = np.zeros((HID, B_L, NP), np.float32)
            out[:, :, 0:N] = a
            return out.reshape(HID, BNP)

        m = dict(common)
        m["xm5"] = xm5
        m["h0i"] = fm(init[0, b0:b0 + B_L])
        m["h1i"] = fm(init[1, b0:b0 + B_L])
        in_maps.append(m)
    return in_maps


def kernel(**inputs):
    from concourse.bass_utils import run_bass_kernel_spmd

    nc = _build()
    in_maps = _host_prep(inputs)
    res = run_bass_kernel_spmd(nc, in_maps, list(range(NCORES)))

    b_fc = float(np.asarray(inputs["b_fc"], np.float32).reshape(-1)[0])
    B = inputs["inputs"].shape[1]
    out = np.zeros((T_STEPS + 1, B, N), np.float32)
    for ci in range(NCORES):
        dev = np.asarray(res.results[ci]["out"])           # [12, 2608]
        dev = dev.reshape(T_STEPS, B_L, NP)[:, :, 0:N]
        out[1:, ci * B_L:(ci + 1) * B_L, :] = dev + b_fc
    return out
